# revision 1
# baseline (speedup 1.0000x reference)
"""AdaptiveMultiWIRE Trainium2 kernel (8 NeuronCores, SPMD data-parallel over
selected channels).

Math per selected channel c (see reference):
  L0: lin = x@w0+b0, lin2 = x@w0o+b0o (real);  h0 = exp(i*30*lin - 100*(lin^2+lin2^2))
  L1: l1 = h0@W1+b1, l2 = h0@W2+b2 (complex, W = (U@V).reshape(181,181))
      h1 = exp(i*30*l1 - 100*(|l1|^2+|l2|^2))
  out = h1@Wf + bf   (complex, (2048, 3))

Device mapping highlights:
  - all matmuls keep the (hid, points) orientation: out = lhsT.T @ rhs with the
    weight stationary, points streaming (N=512 fp32r / fp16).
  - biases folded into matmuls via an appended ones-row on the streaming side
    and a bias row on the stationary side.
  - W1/W2 rows are pre-scaled by 30 and the bias row gets +pi so the L1 psum is
    directly 30*l1re+pi -> feeds sin/cos via DVE python_mod range reduction and
    ACT Sin; squares recover unscaled values via ACT Square(psum/30 + bias).
  - complex arithmetic via 6 real stationary tiles per channel:
    W1re, W1im, W1imneg, W2re, W2im, W2imneg (fp16), built on device from the
    low-rank U@V factorization with a single streaming pass over V.
  - ACT table phases (Sin vs Exp tables cost ~2.7us per switch) are grouped:
    per channel-group: [L0 exp-phase][L0 trig-phase][L1 exp-phase][L1 trig-phase]
    enforced with add_dep_helper barriers on the ACT stream.
"""

import numpy as np

NCORES = 8
NCH, NSEL, NPTS, INF, HID, OUT = 128, 64, 2048, 2, 181, 3
C = NSEL // NCORES  # channels per core
HH, HL = 128, HID - 128  # 128 / 53
KL = HL + 1  # 54 rows in lo K tile (incl ones/bias row)
PT = 512
NT = NPTS // PT
PI = float(np.pi)
OM = 30.0
GROUP_SIZE = 2
VCHUNK = 181 * 32  # 5792
PSUB = 181 * 16  # 2896

_CACHE = {}


def _build(use_r2_mod=True):
    import concourse.bass as bass
    from concourse import bacc
    import concourse.mybir as mybir
    import concourse.tile as tile
    from concourse.tile import add_dep_helper
    from concourse.masks import make_identity

    dt = mybir.dt
    AF = mybir.ActivationFunctionType
    ALU = mybir.AluOpType
    F32, F16, I32 = dt.float32, dt.float16, dt.int32
    F32R = dt.float32r

    nc = bacc.Bacc("TRN2", target_bir_lowering=False, debug=False, num_devices=NCORES)

    # ---------------- DRAM parameters ----------------
    xpk = [nc.declare_dram_parameter(f"xpk{j}", [67, NPTS], F32R, isOutput=False)
           for j in range(3)]
    idx = nc.declare_dram_parameter("idx", [C, 1], I32, isOutput=False)
    w0pack = nc.declare_dram_parameter("w0pack", [NCH, 3 * 368], F32, isOutput=False)
    upack = nc.declare_dram_parameter("upack", [NCH, 16], F32, isOutput=False)
    vpack = nc.declare_dram_parameter("vpack", [16, HID * HID + 1], F32R, isOutput=False)
    b1pack = nc.declare_dram_parameter("b1pack", [NCH, 6 * HID], F32, isOutput=False)
    wfpack = nc.declare_dram_parameter("wfpack", [NCH, 2 * 182 * 6], F32, isOutput=False)
    out48 = nc.declare_dram_parameter("out48", [6 * C, NPTS], F32, isOutput=True)

    acts_by_phase = []  # list of lists of BassInstruction (ACT ops) per table phase
    cur_acts = None

    def act(op):
        cur_acts.append(op)
        return op

    with tile.TileContext(nc) as tc:
        with (
            tc.tile_pool(name="cpool", bufs=1) as cpool,
            tc.tile_pool(name="wpool", bufs=1) as wpool,
        ):
            # ---------- constants ----------
            def constv(val):
                t = cpool.tile([128, 1], F32, tag=f"const{val}", name=f"const{val}".replace(".","_").replace("-","m"))
                nc.vector.memset(t[:], float(val))
                return t

            c_negpi30 = constv(-PI / 30.0)
            c_015 = constv(0.15)
            c_225 = constv(2.25)
            bar_a = cpool.tile([1, 1], F32, tag="bar_a")
            bar_b = cpool.tile([1, 1], F32, tag="bar_b")
            nc.vector.memset(bar_a[:], 0.0)
            nc.vector.memset(bar_b[:], 0.0)

            with (
                tc.tile_pool(name="gpool", bufs=1) as gpool,
                tc.tile_pool(name="vpool", bufs=1) as vpool,
                tc.tile_pool(name="stg", bufs=1) as stgpool,
                tc.tile_pool(name="dstage", bufs=1, space="DRAM") as dpool,
                tc.tile_pool(name="psW", bufs=1, space="PSUM") as psW,
            ):
                # ---------- prologue: gathers ----------
                idx_t = gpool.tile([C, 1], I32)
                nc.sync.dma_start(idx_t[:], idx[:])

                def gather(table, width):
                    t = gpool.tile([C, width], F32)
                    nc.gpsimd.indirect_dma_start(
                        out=t[:], out_offset=None, in_=table[:],
                        in_offset=bass.IndirectOffsetOnAxis(ap=idx_t[:, :1], axis=0),
                    )
                    return t

                w0g = gather(w0pack, 3 * 368)
                upg = gather(upack, 16)
                b1g = gather(b1pack, 6 * HID)
                wfg = gather(wfpack, 2 * 182 * 6)

                # ---------- x and w0 stationary tiles ----------
                xsb = [cpool.tile([67, NPTS], F32R, tag=f"xsb{j}", name=f"xsb{j}") for j in range(3)]
                for j in range(3):
                    nc.sync.dma_start(xsb[j][:], xpk[j][:])
                w0sb = [cpool.tile([67, 368], F32R, tag=f"w0sb{j}", name=f"w0sb{j}") for j in range(3)]
                nc.vector.tensor_scalar_mul(w0g[:], w0g[:], OM)
                nc.vector.tensor_scalar_add(w0g[:, 736:917], w0g[:, 736:917], PI)
                w0g_d = dpool.tile([C, 3 * 368], F32)
                nc.sync.dma_start(w0g_d[:], w0g[:])
                for ch in range(C):
                    j, s = ch // 3, ch % 3
                    nc.gpsimd.dma_start(
                        w0sb[j][32 * s:32 * s + 3, :],
                        w0g_d[ch:ch + 1, :].rearrange("p (a b) -> (p a) b", a=3),
                    )

                # ---------- U lhsT for W-build ----------
                upneg = gpool.tile([C, 16], F32)
                nc.vector.tensor_scalar_mul(upneg[:], upg[:], -OM)
                nc.vector.tensor_scalar_mul(upg[:], upg[:], OM)
                ident8 = cpool.tile([C, C], F32, tag="ident8")
                make_identity(nc, ident8[:])
                upT_ps = psW.tile([16, C], F32, space="PSUM", tag="upT")
                nc.tensor.transpose(upT_ps[:], upg[:], ident8[:])
                upT = gpool.tile([16, C], F32)
                nc.vector.tensor_copy(upT[:], upT_ps[:])
                upnT_ps = psW.tile([16, C], F32, space="PSUM", tag="upT")
                nc.tensor.transpose(upnT_ps[:], upneg[:], ident8[:])
                upnT = gpool.tile([16, C], F32)
                nc.vector.tensor_copy(upnT[:], upnT_ps[:])

                ulhsT = cpool.tile([16, 6 * C], F32R, tag="ulhsT")
                zfill = gpool.tile([16, 6 * C], F32)
                nc.vector.memset(zfill[:], 0.0)
                nc.gpsimd.dma_start(ulhsT[:], zfill[:])
                # block spec: (m, dst_rows_start, src_tensor, src_rows_start)
                blocks = [
                    (0, 0, upT, 0), (0, 4, upnT, 4),
                    (1, 0, upT, 4), (1, 4, upT, 0),
                    (2, 0, upnT, 4), (2, 4, upnT, 0),
                    (3, 8, upT, 8), (3, 12, upnT, 12),
                    (4, 8, upT, 12), (4, 12, upT, 8),
                    (5, 8, upnT, 12), (5, 12, upnT, 8),
                ]
                for m, dr, srct, sr in blocks:
                    nc.gpsimd.dma_start(ulhsT[dr:dr + 4, 8 * m:8 * m + 8],
                                        srct[sr:sr + 4, :])

                # ---------- W tiles ----------
                # per (ch, m): hi (128, 181) fp16 and lo (54, 181) fp16
                Whi = [[wpool.tile([HH, HID], F16, tag=f"Whi{ch}_{m}", name=f"Whi{ch}_{m}") for m in range(6)]
                       for ch in range(C)]
                Wlo = [[wpool.tile([KL, HID], F16, tag=f"Wlo{ch}_{m}", name=f"Wlo{ch}_{m}") for m in range(6)]
                       for ch in range(C)]
                
                # W-build: 3 V chunks of 64 k-rows (last 53)
                kchunks = [(0, 32), (32, 32), (64, 32), (96, 32), (128, 32), (160, 21)]
                for ci, (k0, nk) in enumerate(kchunks):
                    F = nk * HID
                    Fm = F + (F & 1)
                    base = k0 * HID
                    vch = vpool.tile([16, VCHUNK], F32R, tag="vch")
                    nc.sync.dma_start(vch[:, :Fm], vpack[:, base:base + Fm])
                    stg = stgpool.tile([6 * C, VCHUNK], F16, tag="wstg")
                    stg_d = dpool.tile([6 * C, VCHUNK], F16, tag="wstg_d", name="stg_d")
                    nsub = (Fm + PSUB - 1) // PSUB
                    for sub in range(nsub):
                        o0 = sub * PSUB
                        Fs = min(PSUB, Fm - o0)
                        ps = psW.tile([6 * C, PSUB], F32, space="PSUM", tag="wps")
                        for o in range(0, Fs, PT):
                            w = min(PT, Fs - o)
                            nc.tensor.matmul(ps[:, o:o + w], ulhsT[:],
                                             vch[:, o0 + o:o0 + o + w],
                                             start=True, stop=True)
                        h = (Fs // 2 + 3) & ~3
                        nc.vector.tensor_copy(stg[:, o0:o0 + h], ps[:, 0:h])
                        nc.scalar.activation(stg[:, o0 + h:o0 + Fs], ps[:, h:Fs], AF.Copy)
                    nc.sync.dma_start(stg_d[:, :F], stg[:, :F])
                    for ch in range(C):
                        for m in range(6):
                            row = 8 * m + ch
                            src = stg_d[row:row + 1, :F].rearrange("p (k g) -> (p k) g", k=nk)
                            if k0 < HH:
                                nc.sync.dma_start(Whi[ch][m][k0:k0 + nk, :], src)
                            else:
                                nc.sync.dma_start(Wlo[ch][m][k0 - HH:k0 - HH + nk, :], src)

                # bias rows into Wlo row HL
                nc.vector.tensor_scalar_mul(b1g[:], b1g[:], OM)
                nc.vector.tensor_scalar_add(b1g[:, 0:HID], b1g[:, 0:HID], PI)
                for ch in range(C):
                    for m in range(6):
                        nc.gpsimd.dma_start(Wlo[ch][m][HL:KL, :],
                                            b1g[ch:ch + 1, m * HID:(m + 1) * HID])

                # ---------- Wf tiles ----------
                WfHi = [[wpool.tile([HH, 6], F16, tag=f"Wfh{ch}_{a}", name=f"Wfh{ch}_{a}") for a in range(2)]
                        for ch in range(C)]
                WfLo = [[wpool.tile([KL, 6], F16, tag=f"Wfl{ch}_{a}", name=f"Wfl{ch}_{a}") for a in range(2)]
                        for ch in range(C)]
                wfg_d = dpool.tile([C, 2 * 182 * 6], F32)
                nc.sync.dma_start(wfg_d[:], wfg[:])
                for ch in range(C):
                    for a in range(2):
                        o = a * 182 * 6
                        nc.gpsimd.dma_start(
                            WfHi[ch][a][:],
                            wfg_d[ch:ch + 1, o:o + HH * 6].rearrange("p (k g) -> (p k) g", k=HH))
                        nc.gpsimd.dma_start(
                            WfLo[ch][a][:],
                            wfg_d[ch:ch + 1, o + HH * 6:o + 182 * 6].rearrange(
                                "p (k g) -> (p k) g", k=KL))


            tc.strict_bb_all_engine_barrier()
            with (
                tc.tile_pool(name="planes", bufs=2) as pl2,
                tc.tile_pool(name="planes1", bufs=2) as pl1,
                tc.tile_pool(name="fpool", bufs=1) as fpool,
                tc.tile_pool(name="pmain", bufs=1, space="PSUM") as pmain,
            ):
                # ---------- main pipeline ----------
                ngroups = C // GROUP_SIZE

                def new_phase():
                    nonlocal cur_acts
                    cur_acts = []
                    acts_by_phase.append(cur_acts)

                def plane(pool, tag, rows, dtp=F16, bufs=None):
                    return pool.tile([rows, NPTS], dtp, tag=tag, name=tag, bufs=bufs)

                hplanes = {}

                for g in range(ngroups):
                    chans = list(range(g * GROUP_SIZE, (g + 1) * GROUP_SIZE))

                    # ======== L0 EXP phase ========
                    new_phase()
                    grp = {}
                    for ch in chans:
                        j, s = ch // 3, ch % 3
                        b = 32 * s
                        qacc_h = plane(pl2, "qacc_h", HH, bufs=1)
                        qacc_l = plane(pl2, "qacc_l", HL, bufs=1)
                        qtmp_h = plane(pl2, "qtmp_h", HH, bufs=1)
                        qtmp_l = plane(pl2, "qtmp_l", HL, bufs=1)
                        r1_h = plane(pl2, "r1_h", HH)
                        r1_l = plane(pl2, "r1_l", HL)
                        e_h = plane(pl2, "e_h", HH)
                        e_l = plane(pl2, "e_l", HL)
                        for nt in range(NT):
                            sl = slice(nt * PT, (nt + 1) * PT)
                            p_h = pmain.tile([HH, PT], F32, space="PSUM", tag="pa", bufs=2, name="pa")
                            p_l = pmain.tile([KL, PT], F32, space="PSUM", tag="pb", bufs=1, name="pb")
                            p2_h = pmain.tile([HH, PT], F32, space="PSUM", tag="pc", bufs=2, name="pc")
                            p2_l = pmain.tile([KL, PT], F32, space="PSUM", tag="pd", bufs=1, name="pd")
                            rhs = xsb[j][b:b + 3, sl]
                            lh = w0sb[j][b:b + 3, :]
                            nc.tensor.matmul(p_h[:], lh[:, 0:HH], rhs, start=True, stop=True)
                            nc.tensor.matmul(p_l[:], lh[:, HH:182], rhs, start=True, stop=True)
                            nc.tensor.matmul(p2_h[:], lh[:, 184:312], rhs, start=True, stop=True)
                            nc.tensor.matmul(p2_l[:], lh[:, 312:366], rhs, start=True, stop=True)
                            # squares (ACT, exp-table phase)
                            act(nc.scalar.activation(qacc_h[:, sl], p_h[:], AF.Square,
                                                     scale=1.0 / OM, bias=c_negpi30[:HH, :1]))
                            act(nc.scalar.activation(qacc_l[:, sl], p_l[0:HL, :], AF.Square,
                                                     scale=1.0 / OM, bias=c_negpi30[:HL, :1]))
                            act(nc.scalar.activation(qtmp_h[:, sl], p2_h[:], AF.Square,
                                                     scale=1.0 / OM))
                            act(nc.scalar.activation(qtmp_l[:, sl], p2_l[0:HL, :], AF.Square,
                                                     scale=1.0 / OM))
                            # range reductions (DVE)
                            nc.vector.add_range_wrap(r1_h[:, sl], p_h[:], -PI, 2 * PI, 4 * PI)
                            nc.vector.add_range_wrap(r1_l[:, sl], p_l[0:HL, :], -PI, 2 * PI, 4 * PI)
                            nc.vector.add_range_wrap(r1_h[:, sl], r1_h[:, sl], 0.0, PI, 2 * PI)
                            nc.vector.add_range_wrap(r1_l[:, sl], r1_l[:, sl], 0.0, PI, 2 * PI)
                            nc.vector.tensor_add(qacc_h[:, sl], qacc_h[:, sl], qtmp_h[:, sl])
                            nc.vector.tensor_add(qacc_l[:, sl], qacc_l[:, sl], qtmp_l[:, sl])
                        act(nc.scalar.activation(e_h[:], qacc_h[:], AF.Exp, scale=-100.0))
                        act(nc.scalar.activation(e_l[:], qacc_l[:], AF.Exp, scale=-100.0))
                        grp[ch] = (r1_h, r1_l, e_h, e_l)

                    # ======== L0 TRIG phase ========
                    new_phase()
                    for ch in chans:
                        r1_h, r1_l, e_h, e_l = grp[ch]
                        s_h = plane(pl2, "s_h", HH, bufs=1)
                        s_l = plane(pl2, "s_l", HL, bufs=1)
                        co_h = plane(pl2, "co_h", HH, bufs=1)
                        co_l = plane(pl2, "co_l", HL, bufs=1)
                        r2_h = plane(pl2, "r2_h", HH, bufs=1)
                        r2_l = plane(pl2, "r2_l", HL, bufs=1)
                        nc.vector.add_range_wrap(r2_h[:], r1_h[:], PI / 2, PI, 2 * PI)
                        nc.vector.add_range_wrap(r2_l[:], r1_l[:], PI / 2, PI, 2 * PI)
                        h0re_h = plane(pl1, "h0re_h", HH)
                        h0re_l = plane(pl1, "h0re_l", KL)
                        h0im_h = plane(pl1, "h0im_h", HH)
                        h0im_l = plane(pl1, "h0im_l", KL)
                        nc.vector.memset(h0re_l[:], 1.0)
                        nc.vector.memset(h0im_l[:], 0.0)
                        act(nc.scalar.activation(s_h[:], r1_h[:], AF.Sin))
                        act(nc.scalar.activation(s_l[:], r1_l[:], AF.Sin))
                        act(nc.scalar.activation(co_h[:], r2_h[:], AF.Sin))
                        act(nc.scalar.activation(co_l[:], r2_l[:], AF.Sin))
                        nc.vector.tensor_mul(h0re_h[:], e_h[:], co_h[:])
                        nc.vector.tensor_mul(h0re_l[0:HL, :], e_l[:], co_l[:])
                        nc.vector.tensor_mul(h0im_h[:], e_h[:], s_h[:])
                        nc.vector.tensor_mul(h0im_l[0:HL, :], e_l[:], s_l[:])
                        hplanes[ch] = (h0re_h, h0re_l, h0im_h, h0im_l)

                    # ======== L1 EXP phase ========
                    new_phase()
                    grp1 = {}
                    for ch in chans:
                        h0re_h, h0re_l, h0im_h, h0im_l = hplanes[ch]
                        qacc_h = plane(pl2, "qacc_h", HH, bufs=1)
                        qacc_l = plane(pl2, "qacc_l", HL, bufs=1)
                        qtmp_h = plane(pl2, "qtmp_h", HH, bufs=1)
                        qtmp_l = plane(pl2, "qtmp_l", HL, bufs=1)
                        r1_h = plane(pl2, "r1_h", HH)
                        r1_l = plane(pl2, "r1_l", HL)
                        e_h = plane(pl2, "e_h", HH)
                        e_l = plane(pl2, "e_l", HL)

                        def mm4(psum, mstat, msl, rhs_sl):
                            # accumulate 4 terms: (Whi,h0re_h),(Wlo,h0re_l),(Whi',h0im_h),(Wlo',h0im_l)
                            m_a, m_b = mstat
                            nc.tensor.matmul(psum, Whi[ch][m_a][:, msl], h0re_h[:, rhs_sl], start=True, stop=False)
                            nc.tensor.matmul(psum, Wlo[ch][m_a][:, msl], h0re_l[:, rhs_sl], start=False, stop=False)
                            nc.tensor.matmul(psum, Whi[ch][m_b][:, msl], h0im_h[:, rhs_sl], start=False, stop=False)
                            nc.tensor.matmul(psum, Wlo[ch][m_b][:, msl], h0im_l[:, rhs_sl], start=False, stop=True)

                        for nt in range(NT):
                            sl = slice(nt * PT, (nt + 1) * PT)
                            # pair A: l1re (m0 + m2), l1im (m1 + m0)
                            pre_h = pmain.tile([HH, PT], F32, space="PSUM", tag="pa", bufs=2, name="pa")
                            pre_l = pmain.tile([HL, PT], F32, space="PSUM", tag="pb", bufs=1, name="pb")
                            pim_h = pmain.tile([HH, PT], F32, space="PSUM", tag="pc", bufs=2, name="pc")
                            pim_l = pmain.tile([HL, PT], F32, space="PSUM", tag="pd", bufs=1, name="pd")
                            mm4(pre_h[:], (0, 2), slice(0, HH), sl)
                            mm4(pre_l[:], (0, 2), slice(HH, HID), sl)
                            mm4(pim_h[:], (1, 0), slice(0, HH), sl)
                            mm4(pim_l[:], (1, 0), slice(HH, HID), sl)
                            act(nc.scalar.activation(qacc_h[:, sl], pre_h[:], AF.Square,
                                                     scale=1.0 / OM, bias=c_negpi30[:HH, :1]))
                            act(nc.scalar.activation(qacc_l[:, sl], pre_l[:], AF.Square,
                                                     scale=1.0 / OM, bias=c_negpi30[:HL, :1]))
                            act(nc.scalar.activation(qtmp_h[:, sl], pim_h[:], AF.Square,
                                                     scale=1.0 / OM, bias=c_015[:HH, :1]))
                            act(nc.scalar.activation(qtmp_l[:, sl], pim_l[:], AF.Square,
                                                     scale=1.0 / OM, bias=c_015[:HL, :1]))
                            nc.vector.add_range_wrap(r1_h[:, sl], pre_h[:], -PI, 2 * PI, 4 * PI)
                            nc.vector.add_range_wrap(r1_l[:, sl], pre_l[:], -PI, 2 * PI, 4 * PI)
                            nc.vector.add_range_wrap(r1_h[:, sl], r1_h[:, sl], 0.0, PI, 2 * PI)
                            nc.vector.add_range_wrap(r1_l[:, sl], r1_l[:, sl], 0.0, PI, 2 * PI)
                            nc.vector.tensor_add(qacc_h[:, sl], qacc_h[:, sl], qtmp_h[:, sl])
                            nc.vector.tensor_add(qacc_l[:, sl], qacc_l[:, sl], qtmp_l[:, sl])
                            # pair B: l2re (m3 + m5), l2im (m4 + m3)
                            p2re_h = pmain.tile([HH, PT], F32, space="PSUM", tag="pa", bufs=2, name="pa")
                            p2re_l = pmain.tile([HL, PT], F32, space="PSUM", tag="pb", bufs=1, name="pb")
                            p2im_h = pmain.tile([HH, PT], F32, space="PSUM", tag="pc", bufs=2, name="pc")
                            p2im_l = pmain.tile([HL, PT], F32, space="PSUM", tag="pd", bufs=1, name="pd")
                            mm4(p2re_h[:], (3, 5), slice(0, HH), sl)
                            mm4(p2re_l[:], (3, 5), slice(HH, HID), sl)
                            mm4(p2im_h[:], (4, 3), slice(0, HH), sl)
                            mm4(p2im_l[:], (4, 3), slice(HH, HID), sl)
                            act(nc.scalar.activation(qtmp_h[:, sl], p2re_h[:], AF.Square,
                                                     scale=1.0 / OM))
                            act(nc.scalar.activation(qtmp_l[:, sl], p2re_l[:], AF.Square,
                                                     scale=1.0 / OM))
                            nc.vector.tensor_add(qacc_h[:, sl], qacc_h[:, sl], qtmp_h[:, sl])
                            nc.vector.tensor_add(qacc_l[:, sl], qacc_l[:, sl], qtmp_l[:, sl])
                            act(nc.scalar.activation(qtmp_h[:, sl], p2im_h[:], AF.Square,
                                                     scale=1.0 / OM))
                            act(nc.scalar.activation(qtmp_l[:, sl], p2im_l[:], AF.Square,
                                                     scale=1.0 / OM))
                            nc.vector.tensor_add(qacc_h[:, sl], qacc_h[:, sl], qtmp_h[:, sl])
                            nc.vector.tensor_add(qacc_l[:, sl], qacc_l[:, sl], qtmp_l[:, sl])
                        act(nc.scalar.activation(e_h[:], qacc_h[:], AF.Exp,
                                                 scale=-100.0, bias=c_225[:HH, :1]))
                        act(nc.scalar.activation(e_l[:], qacc_l[:], AF.Exp,
                                                 scale=-100.0, bias=c_225[:HL, :1]))
                        grp1[ch] = (r1_h, r1_l, e_h, e_l)

                    # ======== L1 TRIG phase (+ final matmul, PE/DVE only) ========
                    new_phase()
                    for ch in chans:
                        r1_h, r1_l, e_h, e_l = grp1[ch]
                        s_h = plane(pl2, "s_h", HH, bufs=1)
                        s_l = plane(pl2, "s_l", HL, bufs=1)
                        co_h = plane(pl2, "co_h", HH, bufs=1)
                        co_l = plane(pl2, "co_l", HL, bufs=1)
                        r2_h = plane(pl2, "r2_h", HH, bufs=1)
                        r2_l = plane(pl2, "r2_l", HL, bufs=1)
                        nc.vector.add_range_wrap(r2_h[:], r1_h[:], PI / 2, PI, 2 * PI)
                        nc.vector.add_range_wrap(r2_l[:], r1_l[:], PI / 2, PI, 2 * PI)
                        h1re_h = plane(pl1, "h1re_h", HH)
                        h1re_l = plane(pl1, "h1re_l", KL)
                        h1im_h = plane(pl1, "h1im_h", HH)
                        h1im_l = plane(pl1, "h1im_l", KL)
                        nc.vector.memset(h1re_l[:], 1.0)
                        nc.vector.memset(h1im_l[:], 0.0)
                        act(nc.scalar.activation(s_h[:], r1_h[:], AF.Sin))
                        act(nc.scalar.activation(s_l[:], r1_l[:], AF.Sin))
                        act(nc.scalar.activation(co_h[:], r2_h[:], AF.Sin))
                        act(nc.scalar.activation(co_l[:], r2_l[:], AF.Sin))
                        nc.vector.tensor_mul(h1re_h[:], e_h[:], co_h[:])
                        nc.vector.tensor_mul(h1re_l[0:HL, :], e_l[:], co_l[:])
                        nc.vector.tensor_mul(h1im_h[:], e_h[:], s_h[:])
                        nc.vector.tensor_mul(h1im_l[0:HL, :], e_l[:], s_l[:])
                        # final layer: out (6, 2048) += per-nt matmuls
                        for nt in range(NT):
                            sl = slice(nt * PT, (nt + 1) * PT)
                            pf = pmain.tile([6, PT], F32, space="PSUM", tag="fin", bufs=2, name="pf")
                            nc.tensor.matmul(pf[:], WfHi[ch][0][:], h1re_h[:, sl], start=True, stop=False)
                            nc.tensor.matmul(pf[:], WfLo[ch][0][:], h1re_l[:, sl], start=False, stop=False)
                            nc.tensor.matmul(pf[:], WfHi[ch][1][:], h1im_h[:, sl], start=False, stop=False)
                            nc.tensor.matmul(pf[:], WfLo[ch][1][:], h1im_l[:, sl], start=False, stop=True)
                            fs = fpool.tile([6, PT], F32, tag="fstage")
                            nc.vector.tensor_copy(fs[:], pf[:])
                            nc.sync.dma_start(out48[6 * ch:6 * ch + 6, sl], fs[:])


            # ---------- ACT phase barriers ----------
            bars = []
            for p in range(0):
                if p % 2 == 0:
                    b = nc.scalar.copy(bar_b[:], bar_a[:])
                else:
                    b = nc.scalar.copy(bar_a[:], bar_b[:])
                bars.append(b)
            for p, b in enumerate(bars):
                for op in acts_by_phase[p]:
                    add_dep_helper(op.ins, b.ins, sync=False, reason=f"phase{p}end")
                for op in acts_by_phase[p + 1]:
                    add_dep_helper(b.ins, op.ins, sync=False, reason=f"phase{p+1}start")

    nc.compile()
    return nc


def _prep(inputs):
    x = np.ascontiguousarray(inputs["x"], dtype=np.float32)
    indices = np.ascontiguousarray(inputs["indices"], dtype=np.int32)
    w0_lin = np.asarray(inputs["w0_lin"], dtype=np.float32)
    b0_lin = np.asarray(inputs["b0_lin"], dtype=np.float32)
    w0_orth = np.asarray(inputs["w0_orth"], dtype=np.float32)
    b0_orth = np.asarray(inputs["b0_orth"], dtype=np.float32)
    U1_lin = np.asarray(inputs["U1_lin"], dtype=np.complex64)
    V1_lin = np.asarray(inputs["V1_lin"], dtype=np.complex64)
    b1_lin = np.asarray(inputs["b1_lin"], dtype=np.complex64)
    U1_orth = np.asarray(inputs["U1_orth"], dtype=np.complex64)
    V1_orth = np.asarray(inputs["V1_orth"], dtype=np.complex64)
    b1_orth = np.asarray(inputs["b1_orth"], dtype=np.complex64)
    Wf = np.asarray(inputs["Wf"], dtype=np.complex64)
    bf = np.asarray(inputs["bf"], dtype=np.complex64)

    # w0pack rows: per channel 3x362 flattened: rows = [w0x | w0ox ; w0y | w0oy ; b0 | b0o]
    w0pack = np.zeros((NCH, 3, 368), np.float32)
    w0pack[:, 0:2, 0:HID] = w0_lin
    w0pack[:, 2, 0:HID] = b0_lin[:, 0, :]
    w0pack[:, 0:2, 184:184 + HID] = w0_orth
    w0pack[:, 2, 184:184 + HID] = b0_orth[:, 0, :]
    w0pack = w0pack.reshape(NCH, 3 * 368)

    upack = np.concatenate([U1_lin.real, U1_lin.imag, U1_orth.real, U1_orth.imag],
                           axis=1).astype(np.float32)  # (128, 16)
    vpack = np.concatenate([V1_lin.real, V1_lin.imag, V1_orth.real, V1_orth.imag],
                           axis=0).astype(np.float32)  # (16, 32761)
    vpack = np.concatenate([vpack, np.zeros((16, 1), np.float32)], axis=1)
    z = np.zeros_like(b1_lin[:, 0, :].real)
    b1pack = np.concatenate([b1_lin[:, 0, :].real, b1_lin[:, 0, :].imag, z,
                             b1_orth[:, 0, :].real, b1_orth[:, 0, :].imag, z],
                            axis=1).astype(np.float32)  # (128, 1086)

    wfpack = np.zeros((NCH, 2, 182, 6), np.float32)
    wfpack[:, 0, 0:HID, 0:3] = Wf.real
    wfpack[:, 0, 0:HID, 3:6] = Wf.imag
    wfpack[:, 0, HID, 0:3] = bf[:, 0, :].real
    wfpack[:, 0, HID, 3:6] = bf[:, 0, :].imag
    wfpack[:, 1, 0:HID, 0:3] = -Wf.imag
    wfpack[:, 1, 0:HID, 3:6] = Wf.real
    wfpack = wfpack.reshape(NCH, 2 * 182 * 6)

    in_maps = []
    for core in range(NCORES):
        c0 = core * C
        xs = x[c0:c0 + C]  # (8, 2048, 2)
        xpk = [np.zeros((67, NPTS), np.float32) for _ in range(3)]
        for ch in range(C):
            j, s = ch // 3, ch % 3
            xpk[j][32 * s:32 * s + 2, :] = xs[ch].T
            xpk[j][32 * s + 2, :] = 1.0
        m = {f"xpk{j}": xpk[j] for j in range(3)}
        m["idx"] = indices[c0:c0 + C].reshape(C, 1)
        m["w0pack"] = w0pack
        m["upack"] = upack
        m["vpack"] = vpack
        m["b1pack"] = b1pack
        m["wfpack"] = wfpack
        in_maps.append(m)
    return in_maps


def kernel(**inputs):
    from concourse import bass_utils
    if "nc" not in _CACHE:
        _CACHE["nc"] = _build()
    nc = _CACHE["nc"]
    in_maps = _prep(inputs)
    res = bass_utils.run_bass_kernel_spmd(nc, in_maps, core_ids=list(range(NCORES)))
    out = np.zeros((NSEL, NPTS, OUT), np.complex64)
    for core in range(NCORES):
        o = res.results[core]["out48"]  # (48, 2048)
        for ch in range(C):
            re = o[6 * ch:6 * ch + 3, :]  # (3, 2048)
            im = o[6 * ch + 3:6 * ch + 6, :]
            out[core * C + ch] = (re + 1j * im).T.astype(np.complex64)
    return out



# revision 2
# speedup vs baseline: 1.1077x; 1.1077x over previous
"""AdaptiveMultiWIRE Trainium2 kernel (8 NeuronCores, SPMD data-parallel over
selected channels).

Math per selected channel c (see reference):
  L0: lin = x@w0+b0, lin2 = x@w0o+b0o (real);  h0 = exp(i*30*lin - 100*(lin^2+lin2^2))
  L1: l1 = h0@W1+b1, l2 = h0@W2+b2 (complex, W = (U@V).reshape(181,181))
      h1 = exp(i*30*l1 - 100*(|l1|^2+|l2|^2))
  out = h1@Wf + bf   (complex, (2048, 3))

Key structure:
  - all matmuls in (hid, pts) orientation: psum = W_lhsT.T @ h_rhs, N=512.
  - lo rows (hid 128:181) of the h planes are PACKED into one [118, pts] tile:
    rows 0:53 = re_lo, rows 64:117 = im_lo, row 117 = ones (bias row), rows
    53:64 = 1.0 (paired with zero stationary rows - harmless).  This gives
    3 accumulation matmuls per psum instead of 4 (K = 128 re_h + 128 im_h +
    118 packed lo+bias).
  - single-stage range reduction: psum = 30*lin (no pi tricks); sin arg =
    wrap(psum, 0, pi, 2pi); cos arg = wrap(psum, pi/2, pi, 2pi); elements
    whose |arg| exceeds the +-3pi coverage have exp(-100 lin^2) ~ 0 so the
    wrong trig value is annihilated.
  - lo sin/cos args col-packed [53, 2*NPTS] -> one Sin op per (ch, layer).
  - ACT table phases grouped per channel group: exp-set then sin-set.
"""

import numpy as np

NCORES = 8
NCH, NSEL, NPTS, INF, HID, OUT = 128, 64, 2048, 2, 181, 3
C = NSEL // NCORES  # channels per core
HH, HL = 128, HID - 128  # 128 / 53
KP = 118  # packed lo tile rows: 0:53 re_lo, 64:117 im_lo, 117 ones
PT = 512
NT = NPTS // PT
PI = float(np.pi)
OM = 30.0
GROUP_SIZE = 2
VCHUNK = 181 * 32  # 5792
PSUB = 181 * 16  # 2896

_CACHE = {}

# output o -> (m_a, m_b): psum_o = W[m_a].T @ h_re + W[m_b].T @ h_im (+bias of m_a)
# m blocks: 0=30*W1re, 1=30*W1im, 2=-30*W1im, 3=30*W2re, 4=30*W2im, 5=-30*W2im
O_MAP = [(0, 2), (1, 0), (3, 5), (4, 3)]


def _build():
    import concourse.bass as bass
    from concourse import bacc
    import concourse.mybir as mybir
    import concourse.tile as tile
    from concourse.masks import make_identity

    dt = mybir.dt
    AF = mybir.ActivationFunctionType
    F32, F16, I32 = dt.float32, dt.float16, dt.int32
    F32R = dt.float32r

    nc = bacc.Bacc("TRN2", target_bir_lowering=False, debug=False, num_devices=NCORES)

    # ---------------- DRAM parameters ----------------
    xpk = [nc.declare_dram_parameter(f"xpk{j}", [67, NPTS], F32R, isOutput=False)
           for j in range(3)]
    idx = nc.declare_dram_parameter("idx", [C, 1], I32, isOutput=False)
    w0pack = nc.declare_dram_parameter("w0pack", [NCH, 3 * 368], F32, isOutput=False)
    upack = nc.declare_dram_parameter("upack", [NCH, 16], F32, isOutput=False)
    vpack = nc.declare_dram_parameter("vpack", [16, HID * HID + 1], F32R, isOutput=False)
    b1pack = nc.declare_dram_parameter("b1pack", [NCH, 6 * HID], F32, isOutput=False)
    wfpack = nc.declare_dram_parameter("wfpack", [NCH, 2 * 182 * 6], F32, isOutput=False)
    out48 = nc.declare_dram_parameter("out48", [6 * C, NPTS], F32, isOutput=True)

    with tile.TileContext(nc) as tc:
        with (
            tc.tile_pool(name="cpool", bufs=1) as cpool,
            tc.tile_pool(name="wpool", bufs=1) as wpool,
        ):
            # ---------- constants ----------
            def constv(val):
                t = cpool.tile([128, 1], F32, tag=f"const{val}",
                               name=f"const{val}".replace(".", "_").replace("-", "m"))
                nc.vector.memset(t[:], float(val))
                return t

            c_015 = constv(0.15)
            c_225 = constv(2.25)

            with (
                tc.tile_pool(name="gpool", bufs=1) as gpool,
                tc.tile_pool(name="vpool", bufs=1) as vpool,
                tc.tile_pool(name="stg", bufs=1) as stgpool,
                tc.tile_pool(name="dstage", bufs=1, space="DRAM") as dpool,
                tc.tile_pool(name="psW", bufs=1, space="PSUM") as psW,
            ):
                # ---------- prologue: gathers ----------
                idx_t = gpool.tile([C, 1], I32)
                nc.sync.dma_start(idx_t[:], idx[:])

                def gather(table, width):
                    t = gpool.tile([C, width], F32)
                    nc.gpsimd.indirect_dma_start(
                        out=t[:], out_offset=None, in_=table[:],
                        in_offset=bass.IndirectOffsetOnAxis(ap=idx_t[:, :1], axis=0),
                    )
                    return t

                w0g = gather(w0pack, 3 * 368)
                upg = gather(upack, 16)
                b1g = gather(b1pack, 6 * HID)
                wfg = gather(wfpack, 2 * 182 * 6)

                # ---------- x and w0 stationary tiles ----------
                xsb = [cpool.tile([67, NPTS], F32R, tag=f"xsb{j}", name=f"xsb{j}") for j in range(3)]
                for j in range(3):
                    nc.sync.dma_start(xsb[j][:], xpk[j][:])
                w0sb = [cpool.tile([67, 368], F32R, tag=f"w0sb{j}", name=f"w0sb{j}") for j in range(3)]
                nc.vector.tensor_scalar_mul(w0g[:], w0g[:], OM)
                w0g_d = dpool.tile([C, 3 * 368], F32)
                nc.sync.dma_start(w0g_d[:], w0g[:])
                for ch in range(C):
                    j, s = ch // 3, ch % 3
                    nc.gpsimd.dma_start(
                        w0sb[j][32 * s:32 * s + 3, :],
                        w0g_d[ch:ch + 1, :].rearrange("p (a b) -> (p a) b", a=3),
                    )

                # ---------- U lhsT for W-build ----------
                upneg = gpool.tile([C, 16], F32)
                nc.vector.tensor_scalar_mul(upneg[:], upg[:], -OM)
                nc.vector.tensor_scalar_mul(upg[:], upg[:], OM)
                ident8 = cpool.tile([C, C], F32, tag="ident8")
                make_identity(nc, ident8[:])
                upT_ps = psW.tile([16, C], F32, space="PSUM", tag="upT")
                nc.tensor.transpose(upT_ps[:], upg[:], ident8[:])
                upT = gpool.tile([16, C], F32)
                nc.vector.tensor_copy(upT[:], upT_ps[:])
                upnT_ps = psW.tile([16, C], F32, space="PSUM", tag="upT")
                nc.tensor.transpose(upnT_ps[:], upneg[:], ident8[:])
                upnT = gpool.tile([16, C], F32)
                nc.vector.tensor_copy(upnT[:], upnT_ps[:])

                ulhsT = cpool.tile([16, 6 * C], F32R, tag="ulhsT")
                zfill = gpool.tile([16, 6 * C], F32)
                nc.vector.memset(zfill[:], 0.0)
                nc.gpsimd.dma_start(ulhsT[:], zfill[:])
                # block spec: (m, dst_rows_start, src_tensor, src_rows_start)
                blocks = [
                    (0, 0, upT, 0), (0, 4, upnT, 4),
                    (1, 0, upT, 4), (1, 4, upT, 0),
                    (2, 0, upnT, 4), (2, 4, upnT, 0),
                    (3, 8, upT, 8), (3, 12, upnT, 12),
                    (4, 8, upT, 12), (4, 12, upT, 8),
                    (5, 8, upnT, 12), (5, 12, upnT, 8),
                ]
                for m, dr, srct, sr in blocks:
                    nc.gpsimd.dma_start(ulhsT[dr:dr + 4, 8 * m:8 * m + 8],
                                        srct[sr:sr + 4, :])

                # ---------- W tiles ----------
                # per (ch, m): hi (128, 181) fp16; per (ch, o): packed lo (118, 181)
                Whi = [[wpool.tile([HH, HID], F16, tag=f"Whi{ch}_{m}", name=f"Whi{ch}_{m}")
                        for m in range(6)] for ch in range(C)]
                Wpk = [[wpool.tile([KP, HID], F16, tag=f"Wpk{ch}_{o}", name=f"Wpk{ch}_{o}")
                        for o in range(4)] for ch in range(C)]
                for ch in range(C):
                    for o in range(4):
                        nc.vector.memset(Wpk[ch][o][:], 0.0)

                # W-build: V chunks of 32 k-rows (last 21)
                kchunks = [(0, 32), (32, 32), (64, 32), (96, 32), (128, 32), (160, 21)]
                for ci, (k0, nk) in enumerate(kchunks):
                    F = nk * HID
                    Fm = F + (F & 1)
                    base = k0 * HID
                    vch = vpool.tile([16, VCHUNK], F32R, tag="vch")
                    nc.sync.dma_start(vch[:, :Fm], vpack[:, base:base + Fm])
                    stg = stgpool.tile([6 * C, VCHUNK], F16, tag="wstg")
                    stg_d = dpool.tile([6 * C, VCHUNK], F16, tag="wstg_d", name="stg_d")
                    nsub = (Fm + PSUB - 1) // PSUB
                    for sub in range(nsub):
                        o0 = sub * PSUB
                        Fs = min(PSUB, Fm - o0)
                        ps = psW.tile([6 * C, PSUB], F32, space="PSUM", tag="wps")
                        for o in range(0, Fs, PT):
                            w = min(PT, Fs - o)
                            nc.tensor.matmul(ps[:, o:o + w], ulhsT[:],
                                             vch[:, o0 + o:o0 + o + w],
                                             start=True, stop=True)
                        h = (Fs // 2 + 3) & ~3
                        nc.vector.tensor_copy(stg[:, o0:o0 + h], ps[:, 0:h])
                        nc.scalar.activation(stg[:, o0 + h:o0 + Fs], ps[:, h:Fs], AF.Copy)
                    nc.sync.dma_start(stg_d[:, :F], stg[:, :F])
                    for ch in range(C):
                        for m in range(6):
                            row = 8 * m + ch
                            src = stg_d[row:row + 1, :F].rearrange("p (k g) -> (p k) g", k=nk)
                            if k0 < HH:
                                nc.sync.dma_start(Whi[ch][m][k0:k0 + nk, :], src)
                        if k0 >= HH:
                            # lo rows -> packed tiles
                            for o, (ma, mb) in enumerate(O_MAP):
                                rowa = 8 * ma + ch
                                rowb = 8 * mb + ch
                                d0 = k0 - HH
                                srca = stg_d[rowa:rowa + 1, :F].rearrange(
                                    "p (k g) -> (p k) g", k=nk)
                                srcb = stg_d[rowb:rowb + 1, :F].rearrange(
                                    "p (k g) -> (p k) g", k=nk)
                                nc.sync.dma_start(Wpk[ch][o][d0:d0 + nk, :], srca)
                                nc.sync.dma_start(Wpk[ch][o][64 + d0:64 + d0 + nk, :], srcb)

                # bias rows into Wpk row 117 (b of m_a block; no pi offset)
                nc.vector.tensor_scalar_mul(b1g[:], b1g[:], OM)
                for ch in range(C):
                    for o, (ma, mb) in enumerate(O_MAP):
                        nc.gpsimd.dma_start(Wpk[ch][o][117:118, :],
                                            b1g[ch:ch + 1, ma * HID:(ma + 1) * HID])

                # ---------- Wf tiles ----------
                WfHi = [[wpool.tile([HH, 6], F16, tag=f"Wfh{ch}_{a}", name=f"Wfh{ch}_{a}")
                         for a in range(2)] for ch in range(C)]
                WfPk = [wpool.tile([KP, 6], F16, tag=f"Wfp{ch}", name=f"Wfp{ch}")
                        for ch in range(C)]
                for ch in range(C):
                    nc.vector.memset(WfPk[ch][:], 0.0)
                wfg_d = dpool.tile([C, 2 * 182 * 6], F32)
                nc.sync.dma_start(wfg_d[:], wfg[:])
                for ch in range(C):
                    for a in range(2):
                        o = a * 182 * 6
                        nc.gpsimd.dma_start(
                            WfHi[ch][a][:],
                            wfg_d[ch:ch + 1, o:o + HH * 6].rearrange("p (k g) -> (p k) g", k=HH))
                    # packed lo: rows 0:53 <- block0 rows 128:181; 64:117 <- block1
                    # rows 128:181; row 117 <- block0 row 181 (bias)
                    nc.gpsimd.dma_start(
                        WfPk[ch][0:HL, :],
                        wfg_d[ch:ch + 1, HH * 6:HID * 6].rearrange("p (k g) -> (p k) g", k=HL))
                    nc.gpsimd.dma_start(
                        WfPk[ch][64:64 + HL, :],
                        wfg_d[ch:ch + 1, 182 * 6 + HH * 6:182 * 6 + HID * 6].rearrange(
                            "p (k g) -> (p k) g", k=HL))
                    nc.gpsimd.dma_start(
                        WfPk[ch][117:118, :],
                        wfg_d[ch:ch + 1, HID * 6:182 * 6])

            tc.strict_bb_all_engine_barrier()
            with (
                tc.tile_pool(name="planes", bufs=2) as pl2,
                tc.tile_pool(name="planes1", bufs=2) as pl1,
                tc.tile_pool(name="fpool", bufs=1) as fpool,
                tc.tile_pool(name="pmain", bufs=1, space="PSUM") as pmain,
            ):
                # ---------- main pipeline ----------
                ngroups = C // GROUP_SIZE

                def plane(pool, tag, rows, dtp=F16, bufs=None, cols=NPTS):
                    return pool.tile([rows, cols], dtp, tag=tag, name=tag, bufs=bufs)

                hplanes = {}

                for g in range(ngroups):
                    chans = list(range(g * GROUP_SIZE, (g + 1) * GROUP_SIZE))

                    # ======== L0 EXP phase ========
                    grp = {}
                    for ch in chans:
                        j, s = ch // 3, ch % 3
                        b = 32 * s
                        qacc_h = plane(pl2, "qacc_h", HH, bufs=1)
                        qacc_l = plane(pl2, "qacc_l", HL, bufs=1)
                        qtmp_h = plane(pl2, "qtmp_h", HH, bufs=1)
                        qtmp_l = plane(pl2, "qtmp_l", HL, bufs=1)
                        sarg_h = plane(pl2, "sarg_h", HH)
                        carg_h = plane(pl2, "carg_h", HH)
                        arg_l = plane(pl2, "arg_l", HL, cols=2 * NPTS)
                        e_h = plane(pl2, "e_h", HH)
                        e_l = plane(pl2, "e_l", HL)
                        for nt in range(NT):
                            sl = slice(nt * PT, (nt + 1) * PT)
                            sl2 = slice(NPTS + nt * PT, NPTS + (nt + 1) * PT)
                            p_h = pmain.tile([HH, PT], F32, space="PSUM", tag="pa", bufs=2, name="pa")
                            p_l = pmain.tile([HL, PT], F32, space="PSUM", tag="pb", bufs=1, name="pb")
                            p2_h = pmain.tile([HH, PT], F32, space="PSUM", tag="pc", bufs=2, name="pc")
                            p2_l = pmain.tile([HL, PT], F32, space="PSUM", tag="pd", bufs=1, name="pd")
                            rhs = xsb[j][b:b + 3, sl]
                            lh = w0sb[j][b:b + 3, :]
                            nc.tensor.matmul(p_h[:], lh[:, 0:HH], rhs, start=True, stop=True)
                            nc.tensor.matmul(p_l[:], lh[:, HH:HID], rhs, start=True, stop=True)
                            nc.tensor.matmul(p2_h[:], lh[:, 184:312], rhs, start=True, stop=True)
                            nc.tensor.matmul(p2_l[:], lh[:, 312:312 + HL], rhs, start=True, stop=True)
                            # squares (ACT, exp-table phase); psum = 30*lin
                            nc.scalar.activation(qacc_h[:, sl], p_h[:], AF.Square,
                                                 scale=1.0 / OM)
                            nc.scalar.activation(qacc_l[:, sl], p_l[:], AF.Square,
                                                 scale=1.0 / OM)
                            nc.scalar.activation(qtmp_h[:, sl], p2_h[:], AF.Square,
                                                 scale=1.0 / OM)
                            nc.scalar.activation(qtmp_l[:, sl], p2_l[:], AF.Square,
                                                 scale=1.0 / OM)
                            # range reduction (DVE): sin/cos args
                            nc.vector.add_range_wrap(sarg_h[:, sl], p_h[:], 0.0, PI, 2 * PI)
                            nc.vector.add_range_wrap(carg_h[:, sl], p_h[:], PI / 2, PI, 2 * PI)
                            nc.vector.add_range_wrap(arg_l[:, sl], p_l[:], 0.0, PI, 2 * PI)
                            nc.vector.add_range_wrap(arg_l[:, sl2], p_l[:], PI / 2, PI, 2 * PI)
                            nc.vector.tensor_add(qacc_h[:, sl], qacc_h[:, sl], qtmp_h[:, sl])
                            nc.vector.tensor_add(qacc_l[:, sl], qacc_l[:, sl], qtmp_l[:, sl])
                        nc.scalar.activation(e_h[:], qacc_h[:], AF.Exp, scale=-100.0)
                        nc.scalar.activation(e_l[:], qacc_l[:], AF.Exp, scale=-100.0)
                        grp[ch] = (sarg_h, carg_h, arg_l, e_h, e_l)

                    # ======== L0 TRIG phase ========
                    for ch in chans:
                        sarg_h, carg_h, arg_l, e_h, e_l = grp[ch]
                        s_h = plane(pl2, "s_h", HH, bufs=1)
                        co_h = plane(pl2, "co_h", HH, bufs=1)
                        sc_l = plane(pl2, "sc_l", HL, bufs=1, cols=2 * NPTS)
                        h0re_h = plane(pl1, "h0re_h", HH)
                        h0im_h = plane(pl1, "h0im_h", HH)
                        h0lo = plane(pl1, "h0lo", KP)
                        nc.vector.memset(h0lo[:], 1.0)
                        nc.scalar.activation(s_h[:], sarg_h[:], AF.Sin)
                        nc.scalar.activation(co_h[:], carg_h[:], AF.Sin)
                        nc.scalar.activation(sc_l[:], arg_l[:], AF.Sin)
                        nc.vector.tensor_mul(h0re_h[:], e_h[:], co_h[:])
                        nc.vector.tensor_mul(h0im_h[:], e_h[:], s_h[:])
                        nc.vector.tensor_mul(h0lo[0:HL, :], e_l[:], sc_l[:, NPTS:])
                        nc.vector.tensor_mul(h0lo[64:64 + HL, :], e_l[:], sc_l[:, 0:NPTS])
                        hplanes[ch] = (h0re_h, h0im_h, h0lo)

                    # ======== L1 EXP phase ========
                    grp1 = {}
                    for ch in chans:
                        h0re_h, h0im_h, h0lo = hplanes[ch]
                        qacc_h = plane(pl2, "qacc_h", HH, bufs=1)
                        qacc_l = plane(pl2, "qacc_l", HL, bufs=1)
                        qtmp_h = plane(pl2, "qtmp_h", HH, bufs=1)
                        qtmp_l = plane(pl2, "qtmp_l", HL, bufs=1)
                        sarg_h = plane(pl2, "sarg_h", HH)
                        carg_h = plane(pl2, "carg_h", HH)
                        arg_l = plane(pl2, "arg_l", HL, cols=2 * NPTS)
                        e_h = plane(pl2, "e_h", HH)
                        e_l = plane(pl2, "e_l", HL)

                        def mm3(psum, o, msl, rhs_sl, first):
                            m_a, m_b = O_MAP[o]
                            nc.tensor.matmul(psum, Whi[ch][m_a][:, msl], h0re_h[:, rhs_sl],
                                             start=True, stop=False)
                            nc.tensor.matmul(psum, Whi[ch][m_b][:, msl], h0im_h[:, rhs_sl],
                                             start=False, stop=False)
                            nc.tensor.matmul(psum, Wpk[ch][o][:, msl], h0lo[:, rhs_sl],
                                             start=False, stop=True)

                        for nt in range(NT):
                            sl = slice(nt * PT, (nt + 1) * PT)
                            sl2 = slice(NPTS + nt * PT, NPTS + (nt + 1) * PT)
                            # pair A: l1re (o=0), l1im (o=1)
                            pre_h = pmain.tile([HH, PT], F32, space="PSUM", tag="pa", bufs=2, name="pa")
                            pre_l = pmain.tile([HL, PT], F32, space="PSUM", tag="pb", bufs=1, name="pb")
                            pim_h = pmain.tile([HH, PT], F32, space="PSUM", tag="pc", bufs=2, name="pc")
                            pim_l = pmain.tile([HL, PT], F32, space="PSUM", tag="pd", bufs=1, name="pd")
                            mm3(pre_h[:], 0, slice(0, HH), sl, True)
                            mm3(pre_l[:], 0, slice(HH, HID), sl, True)
                            mm3(pim_h[:], 1, slice(0, HH), sl, True)
                            mm3(pim_l[:], 1, slice(HH, HID), sl, True)
                            nc.scalar.activation(qacc_h[:, sl], pre_h[:], AF.Square,
                                                 scale=1.0 / OM)
                            nc.scalar.activation(qacc_l[:, sl], pre_l[:], AF.Square,
                                                 scale=1.0 / OM)
                            nc.scalar.activation(qtmp_h[:, sl], pim_h[:], AF.Square,
                                                 scale=1.0 / OM, bias=c_015[:HH, :1])
                            nc.scalar.activation(qtmp_l[:, sl], pim_l[:], AF.Square,
                                                 scale=1.0 / OM, bias=c_015[:HL, :1])
                            nc.vector.add_range_wrap(sarg_h[:, sl], pre_h[:], 0.0, PI, 2 * PI)
                            nc.vector.add_range_wrap(carg_h[:, sl], pre_h[:], PI / 2, PI, 2 * PI)
                            nc.vector.add_range_wrap(arg_l[:, sl], pre_l[:], 0.0, PI, 2 * PI)
                            nc.vector.add_range_wrap(arg_l[:, sl2], pre_l[:], PI / 2, PI, 2 * PI)
                            nc.vector.tensor_add(qacc_h[:, sl], qacc_h[:, sl], qtmp_h[:, sl])
                            nc.vector.tensor_add(qacc_l[:, sl], qacc_l[:, sl], qtmp_l[:, sl])
                            # pair B: l2re (o=2), l2im (o=3)
                            p2re_h = pmain.tile([HH, PT], F32, space="PSUM", tag="pa", bufs=2, name="pa")
                            p2re_l = pmain.tile([HL, PT], F32, space="PSUM", tag="pb", bufs=1, name="pb")
                            p2im_h = pmain.tile([HH, PT], F32, space="PSUM", tag="pc", bufs=2, name="pc")
                            p2im_l = pmain.tile([HL, PT], F32, space="PSUM", tag="pd", bufs=1, name="pd")
                            mm3(p2re_h[:], 2, slice(0, HH), sl, True)
                            mm3(p2re_l[:], 2, slice(HH, HID), sl, True)
                            mm3(p2im_h[:], 3, slice(0, HH), sl, True)
                            mm3(p2im_l[:], 3, slice(HH, HID), sl, True)
                            nc.scalar.activation(qtmp_h[:, sl], p2re_h[:], AF.Square,
                                                 scale=1.0 / OM)
                            nc.scalar.activation(qtmp_l[:, sl], p2re_l[:], AF.Square,
                                                 scale=1.0 / OM)
                            nc.vector.tensor_add(qacc_h[:, sl], qacc_h[:, sl], qtmp_h[:, sl])
                            nc.vector.tensor_add(qacc_l[:, sl], qacc_l[:, sl], qtmp_l[:, sl])
                            nc.scalar.activation(qtmp_h[:, sl], p2im_h[:], AF.Square,
                                                 scale=1.0 / OM)
                            nc.scalar.activation(qtmp_l[:, sl], p2im_l[:], AF.Square,
                                                 scale=1.0 / OM)
                            nc.vector.tensor_add(qacc_h[:, sl], qacc_h[:, sl], qtmp_h[:, sl])
                            nc.vector.tensor_add(qacc_l[:, sl], qacc_l[:, sl], qtmp_l[:, sl])
                        nc.scalar.activation(e_h[:], qacc_h[:], AF.Exp,
                                             scale=-100.0, bias=c_225[:HH, :1])
                        nc.scalar.activation(e_l[:], qacc_l[:], AF.Exp,
                                             scale=-100.0, bias=c_225[:HL, :1])
                        grp1[ch] = (sarg_h, carg_h, arg_l, e_h, e_l)

                    # ======== L1 TRIG phase (+ final matmul) ========
                    for ch in chans:
                        sarg_h, carg_h, arg_l, e_h, e_l = grp1[ch]
                        s_h = plane(pl2, "s_h", HH, bufs=1)
                        co_h = plane(pl2, "co_h", HH, bufs=1)
                        sc_l = plane(pl2, "sc_l", HL, bufs=1, cols=2 * NPTS)
                        h1re_h = plane(pl1, "h1re_h", HH)
                        h1im_h = plane(pl1, "h1im_h", HH)
                        h1lo = plane(pl1, "h1lo", KP)
                        nc.vector.memset(h1lo[:], 1.0)
                        nc.scalar.activation(s_h[:], sarg_h[:], AF.Sin)
                        nc.scalar.activation(co_h[:], carg_h[:], AF.Sin)
                        nc.scalar.activation(sc_l[:], arg_l[:], AF.Sin)
                        nc.vector.tensor_mul(h1re_h[:], e_h[:], co_h[:])
                        nc.vector.tensor_mul(h1im_h[:], e_h[:], s_h[:])
                        nc.vector.tensor_mul(h1lo[0:HL, :], e_l[:], sc_l[:, NPTS:])
                        nc.vector.tensor_mul(h1lo[64:64 + HL, :], e_l[:], sc_l[:, 0:NPTS])
                        # final layer: out (6, 2048)
                        for nt in range(NT):
                            sl = slice(nt * PT, (nt + 1) * PT)
                            pf = pmain.tile([6, PT], F32, space="PSUM", tag="fin", bufs=2, name="pf")
                            nc.tensor.matmul(pf[:], WfHi[ch][0][:], h1re_h[:, sl], start=True, stop=False)
                            nc.tensor.matmul(pf[:], WfHi[ch][1][:], h1im_h[:, sl], start=False, stop=False)
                            nc.tensor.matmul(pf[:], WfPk[ch][:], h1lo[:, sl], start=False, stop=True)
                            fs = fpool.tile([6, PT], F32, tag="fstage")
                            nc.vector.tensor_copy(fs[:], pf[:])
                            nc.sync.dma_start(out48[6 * ch:6 * ch + 6, sl], fs[:])

    nc.compile()
    return nc


def _prep(inputs):
    x = np.ascontiguousarray(inputs["x"], dtype=np.float32)
    indices = np.ascontiguousarray(inputs["indices"], dtype=np.int32)
    w0_lin = np.asarray(inputs["w0_lin"], dtype=np.float32)
    b0_lin = np.asarray(inputs["b0_lin"], dtype=np.float32)
    w0_orth = np.asarray(inputs["w0_orth"], dtype=np.float32)
    b0_orth = np.asarray(inputs["b0_orth"], dtype=np.float32)
    U1_lin = np.asarray(inputs["U1_lin"], dtype=np.complex64)
    V1_lin = np.asarray(inputs["V1_lin"], dtype=np.complex64)
    b1_lin = np.asarray(inputs["b1_lin"], dtype=np.complex64)
    U1_orth = np.asarray(inputs["U1_orth"], dtype=np.complex64)
    V1_orth = np.asarray(inputs["V1_orth"], dtype=np.complex64)
    b1_orth = np.asarray(inputs["b1_orth"], dtype=np.complex64)
    Wf = np.asarray(inputs["Wf"], dtype=np.complex64)
    bf = np.asarray(inputs["bf"], dtype=np.complex64)

    # w0pack rows: per channel 3x368: rows = [w0x | w0ox ; w0y | w0oy ; b0 | b0o]
    w0pack = np.zeros((NCH, 3, 368), np.float32)
    w0pack[:, 0:2, 0:HID] = w0_lin
    w0pack[:, 2, 0:HID] = b0_lin[:, 0, :]
    w0pack[:, 0:2, 184:184 + HID] = w0_orth
    w0pack[:, 2, 184:184 + HID] = b0_orth[:, 0, :]
    w0pack = w0pack.reshape(NCH, 3 * 368)

    upack = np.concatenate([U1_lin.real, U1_lin.imag, U1_orth.real, U1_orth.imag],
                           axis=1).astype(np.float32)  # (128, 16)
    vpack = np.concatenate([V1_lin.real, V1_lin.imag, V1_orth.real, V1_orth.imag],
                           axis=0).astype(np.float32)  # (16, 32761)
    vpack = np.concatenate([vpack, np.zeros((16, 1), np.float32)], axis=1)
    # b1pack blocks m=0..5: [b1re, b1im + 4.5/30? no: im bias fold handled by
    # c_015 in Square] -> [b1re, b1im, 0, b2re, b2im, 0]
    z = np.zeros_like(b1_lin[:, 0, :].real)
    b1pack = np.concatenate([b1_lin[:, 0, :].real, b1_lin[:, 0, :].imag, z,
                             b1_orth[:, 0, :].real, b1_orth[:, 0, :].imag, z],
                            axis=1).astype(np.float32)  # (128, 1086)

    wfpack = np.zeros((NCH, 2, 182, 6), np.float32)
    wfpack[:, 0, 0:HID, 0:3] = Wf.real
    wfpack[:, 0, 0:HID, 3:6] = Wf.imag
    wfpack[:, 0, HID, 0:3] = bf[:, 0, :].real
    wfpack[:, 0, HID, 3:6] = bf[:, 0, :].imag
    wfpack[:, 1, 0:HID, 0:3] = -Wf.imag
    wfpack[:, 1, 0:HID, 3:6] = Wf.real
    wfpack = wfpack.reshape(NCH, 2 * 182 * 6)

    in_maps = []
    for core in range(NCORES):
        c0 = core * C
        xs = x[c0:c0 + C]  # (8, 2048, 2)
        xpk = [np.zeros((67, NPTS), np.float32) for _ in range(3)]
        for ch in range(C):
            j, s = ch // 3, ch % 3
            xpk[j][32 * s:32 * s + 2, :] = xs[ch].T
            xpk[j][32 * s + 2, :] = 1.0
        m = {f"xpk{j}": xpk[j] for j in range(3)}
        m["idx"] = indices[c0:c0 + C].reshape(C, 1)
        m["w0pack"] = w0pack
        m["upack"] = upack
        m["vpack"] = vpack
        m["b1pack"] = b1pack
        m["wfpack"] = wfpack
        in_maps.append(m)
    return in_maps


def kernel(**inputs):
    from concourse import bass_utils
    if "nc" not in _CACHE:
        _CACHE["nc"] = _build()
    nc = _CACHE["nc"]
    in_maps = _prep(inputs)
    res = bass_utils.run_bass_kernel_spmd(nc, in_maps, core_ids=list(range(NCORES)))
    out = np.zeros((NSEL, NPTS, OUT), np.complex64)
    for core in range(NCORES):
        o = res.results[core]["out48"]  # (48, 2048)
        for ch in range(C):
            re = o[6 * ch:6 * ch + 3, :]  # (3, 2048)
            im = o[6 * ch + 3:6 * ch + 6, :]
            out[core * C + ch] = (re + 1j * im).T.astype(np.complex64)
    return out


# revision 6
# speedup vs baseline: 1.5607x; 1.4090x over previous
"""AdaptiveMultiWIRE Trainium2 kernel (8 NeuronCores, SPMD data-parallel over
selected channels).

Math per selected channel c (see reference):
  L0: lin = x@w0+b0, lin2 = x@w0o+b0o (real);  h0 = exp(i*30*lin - 100*(lin^2+lin2^2))
  L1: l1 = h0@W1+b1, l2 = h0@W2+b2 (complex, W = (U@V).reshape(181,181))
      h1 = exp(i*30*l1 - 100*(|l1|^2+|l2|^2))
  out = h1@Wf + bf   (complex, (2048, 3))

Key structure:
  - all matmuls in (hid, pts) orientation: psum = W_lhsT.T @ h_rhs, N=512.
  - lo rows (hid 128:181) of the h planes are PACKED into one [118, pts] tile:
    rows 0:53 = re_lo, rows 64:117 = im_lo, row 117 = ones (bias row), rows
    53:64 = 1.0 (paired with zero stationary rows - harmless).  3 accumulation
    matmuls per psum instead of 4.
  - single-stage range reduction: psum = 30*lin; sin arg = wrap(psum, 0, pi,
    2pi); cos arg = wrap(psum, pi/2, pi, 2pi); elements beyond +-3pi coverage
    have exp(-100 lin^2) ~ 0 so the wrong trig value is annihilated.
  - sin/cos args col-packed -> one Sin op per (ch, layer) for hi and for lo.
  - W-build overlapped with group 0's L0 phase (no barrier); U lhsT built via
    DRAM-transpose DMAs (no PE transpose / extra psum bank).
"""

import numpy as np

NCORES = 8
NCH, NSEL, NPTS, INF, HID, OUT = 128, 64, 2048, 2, 181, 3
C = NSEL // NCORES  # channels per core
HH, HL = 128, HID - 128  # 128 / 53
KP = 118  # packed lo tile rows: 0:53 re_lo, 64:117 im_lo, 117 ones
PT = 512
NT = NPTS // PT
PI = float(np.pi)
OM = 30.0
GROUP_SIZE = 2
VCHUNK = 181 * 32  # 5792

_CACHE = {}

# output o -> (m_a, m_b): psum_o = W[m_a].T @ h_re + W[m_b].T @ h_im (+bias of m_a)
# m blocks: 0=30*W1re, 1=30*W1im, 2=-30*W1im, 3=30*W2re, 4=30*W2im, 5=-30*W2im
O_MAP = [(0, 2), (1, 0), (3, 5), (4, 3)]


def _build():
    import concourse.bass as bass
    from concourse import bacc
    import concourse.mybir as mybir
    import concourse.tile as tile

    dt = mybir.dt
    AF = mybir.ActivationFunctionType
    F32, F16, I32 = dt.float32, dt.float16, dt.int32
    F32R = dt.float32r

    nc = bacc.Bacc("TRN2", target_bir_lowering=False, debug=False, num_devices=NCORES)

    # ---------------- DRAM parameters ----------------
    xpk = [nc.declare_dram_parameter(f"xpk{j}", [67, NPTS], F16, isOutput=False)
           for j in range(3)]
    idx = nc.declare_dram_parameter("idx", [C, 1], I32, isOutput=False)
    w0pack = nc.declare_dram_parameter("w0pack", [NCH, 3 * 368], F32, isOutput=False)
    upack = nc.declare_dram_parameter("upack", [NCH, 32], F32, isOutput=False)
    vpack = nc.declare_dram_parameter("vpack", [16, HID * HID + 1], F32R, isOutput=False)
    b1pack = nc.declare_dram_parameter("b1pack", [NCH, 6 * HID], F32, isOutput=False)
    wfpack = nc.declare_dram_parameter("wfpack", [NCH, 2 * 182 * 6], F32, isOutput=False)
    out48 = nc.declare_dram_parameter("out48", [6 * C, NPTS], F32, isOutput=True)

    with tile.TileContext(nc) as tc:
        with (
            tc.tile_pool(name="cpool", bufs=1) as cpool,
            tc.tile_pool(name="wpool", bufs=1) as wpool,
            tc.tile_pool(name="gpool", bufs=1) as gpool,
            tc.tile_pool(name="vpool", bufs=1) as vpool,
            tc.tile_pool(name="stg", bufs=1) as stgpool,
            tc.tile_pool(name="dstage", bufs=1, space="DRAM") as dpool,
            tc.tile_pool(name="planes", bufs=2) as pl2,
            tc.tile_pool(name="planes1", bufs=2) as pl1,
            tc.tile_pool(name="fpool", bufs=1) as fpool,
            tc.tile_pool(name="pmain", bufs=1, space="PSUM") as pmain,
        ):
            # ---------- constants ----------
            def constv(val):
                t = cpool.tile([128, 1], F32, tag=f"const{val}",
                               name=f"const{val}".replace(".", "_").replace("-", "m"))
                nc.vector.memset(t[:], float(val))
                return t

            c_015 = constv(0.15)
            c_225 = constv(2.25)

            # ---------- prologue: gathers (all early; only need idx) ----------
            idx_t = gpool.tile([C, 1], I32)
            nc.sync.dma_start(idx_t[:], idx[:])

            def gather(table, width):
                t = gpool.tile([C, width], F32)
                nc.gpsimd.indirect_dma_start(
                    out=t[:], out_offset=None, in_=table[:],
                    in_offset=bass.IndirectOffsetOnAxis(ap=idx_t[:, :1], axis=0),
                )
                return t

            w0g = gather(w0pack, 3 * 368)
            upg = gather(upack, 32)
            b1g = gather(b1pack, 6 * HID)
            wfg = gather(wfpack, 2 * 182 * 6)

            # ---------- x and w0 stationary tiles ----------
            xsb = [cpool.tile([67, NPTS], F16, tag=f"xsb{j}", name=f"xsb{j}") for j in range(3)]
            for j in range(3):
                nc.sync.dma_start(xsb[j][:], xpk[j][:])
            w0sb = [cpool.tile([67, 368], F16, tag=f"w0sb{j}", name=f"w0sb{j}") for j in range(3)]
            nc.vector.tensor_scalar_mul(w0g[:], w0g[:], OM)
            w0g_d = dpool.tile([C, 3 * 368], F32)
            nc.sync.dma_start(w0g_d[:], w0g[:])
            for ch in range(C):
                j, s = ch // 3, ch % 3
                nc.gpsimd.dma_start(
                    w0sb[j][32 * s:32 * s + 3, :],
                    w0g_d[ch:ch + 1, :].rearrange("p (a b) -> (p a) b", a=3),
                )

            # ---------- U lhsT via DRAM transpose DMAs ----------
            # upg cols: [U1re U1im U2re U2im | same negated] (host provides both)
            nc.vector.tensor_scalar_mul(upg[:], upg[:], OM)
            upg_d = dpool.tile([C, 32], F32)
            nc.sync.dma_start(upg_d[:], upg[:])
            ulhsT = cpool.tile([16, 6 * C], F32R, tag="ulhsT")
            zfill = gpool.tile([16, 6 * C], F32)
            nc.vector.memset(zfill[:], 0.0)
            nc.gpsimd.dma_start(ulhsT[:], zfill[:])
            # block spec: (m, dst_rows_start, src_col_start) with cols 0:16 = +U,
            # 16:32 = -U; sub-blocks of 4 (r1re, r1im, r2re, r2im)
            blocks = [
                (0, 0, 0), (0, 4, 16 + 4),
                (1, 0, 4), (1, 4, 0),
                (2, 0, 16 + 4), (2, 4, 16 + 0),
                (3, 8, 8), (3, 12, 16 + 12),
                (4, 8, 12), (4, 12, 8),
                (5, 8, 16 + 12), (5, 12, 16 + 8),
            ]
            for m, dr, sc in blocks:
                nc.gpsimd.dma_start(
                    ulhsT[dr:dr + 4, 8 * m:8 * m + 8],
                    upg_d[:, sc:sc + 4].rearrange("p f -> f p"),
                )

            # ---------- W tile declarations ----------
            Whi = [[wpool.tile([HH, HID], F16, tag=f"Whi{ch}_{m}", name=f"Whi{ch}_{m}")
                    for m in range(6)] for ch in range(C)]
            Wpk = [[wpool.tile([KP, HID], F16, tag=f"Wpk{ch}_{o}", name=f"Wpk{ch}_{o}")
                    for o in range(4)] for ch in range(C)]
            for ch in range(C):
                for o in range(4):
                    nc.vector.memset(Wpk[ch][o][:], 0.0)
            WfHi = [[wpool.tile([HH, 6], F16, tag=f"Wfh{ch}_{a}", name=f"Wfh{ch}_{a}")
                     for a in range(2)] for ch in range(C)]
            WfPk = [wpool.tile([KP, 6], F16, tag=f"Wfp{ch}", name=f"Wfp{ch}")
                    for ch in range(C)]
            for ch in range(C):
                nc.vector.memset(WfPk[ch][:], 0.0)

            def emit_wbuild():
                """W = U@V build: psums -> fp16 stage -> DRAM -> lhsT tiles."""
                kchunks = [(0, 32), (32, 32), (64, 32), (96, 32), (128, 32), (160, 21)]
                stg_d = dpool.tile([6 * C, 6 * VCHUNK], F16, tag="wstg_d", name="stg_d")
                for ci, (k0, nk) in enumerate(kchunks):
                    F = nk * HID
                    Fm = F + (F & 1)
                    base = k0 * HID
                    vch = vpool.tile([16, VCHUNK], F32R, tag="vch")
                    nc.sync.dma_start(vch[:, :Fm], vpack[:, base:base + Fm])
                    stg = stgpool.tile([6 * C, VCHUNK], F16, tag="wstg")
                    nsub = (Fm + PT - 1) // PT
                    for sub in range(nsub):
                        o0 = sub * PT
                        Fs = min(PT, Fm - o0)
                        Fs2 = Fs - (Fs & 3)
                        ps = pmain.tile([6 * C, PT], F32, space="PSUM", tag="wps",
                                        bufs=1, name="wps")
                        nc.tensor.matmul(ps[:, :Fs], ulhsT[:],
                                         vch[:, o0:o0 + Fs], start=True, stop=True)
                        if sub % 2 == 0:
                            nc.vector.tensor_copy(stg[:, o0:o0 + Fs2], ps[:, 0:Fs2])
                            if Fs2 < Fs:
                                nc.scalar.activation(stg[:, o0 + Fs2:o0 + Fs],
                                                     ps[:, Fs2:Fs], AF.Copy)
                        else:
                            nc.scalar.activation(stg[:, o0:o0 + Fs], ps[:, 0:Fs], AF.Copy)
                    nc.sync.dma_start(stg_d[:, ci * VCHUNK:ci * VCHUNK + F], stg[:, :F])
                # whole-k rearrange DMAs (stg_d cols are globally (k,j)-contiguous)
                for ch in range(C):
                    for m in range(6):
                        row = 8 * m + ch
                        src = stg_d[row:row + 1, :HID * HID].rearrange(
                            "p (k g) -> (p k) g", k=HID)
                        nc.sync.dma_start(Whi[ch][m][:, :], src[0:HH, :])
                    for o, (ma, mb) in enumerate(O_MAP):
                        rowa = 8 * ma + ch
                        rowb = 8 * mb + ch
                        srca = stg_d[rowa:rowa + 1, :HID * HID].rearrange(
                            "p (k g) -> (p k) g", k=HID)
                        srcb = stg_d[rowb:rowb + 1, :HID * HID].rearrange(
                            "p (k g) -> (p k) g", k=HID)
                        nc.sync.dma_start(Wpk[ch][o][0:HL, :], srca[HH:HID, :])
                        nc.sync.dma_start(Wpk[ch][o][64:64 + HL, :], srcb[HH:HID, :])

                # bias rows into Wpk row 117 (b of m_a block)
                nc.vector.tensor_scalar_mul(b1g[:], b1g[:], OM)
                for ch in range(C):
                    for o, (ma, mb) in enumerate(O_MAP):
                        nc.gpsimd.dma_start(Wpk[ch][o][117:118, :],
                                            b1g[ch:ch + 1, ma * HID:(ma + 1) * HID])

                # Wf tiles
                wfg_d = dpool.tile([C, 2 * 182 * 6], F32)
                nc.sync.dma_start(wfg_d[:], wfg[:])
                for ch in range(C):
                    for a in range(2):
                        o = a * 182 * 6
                        nc.gpsimd.dma_start(
                            WfHi[ch][a][:],
                            wfg_d[ch:ch + 1, o:o + HH * 6].rearrange("p (k g) -> (p k) g", k=HH))
                    nc.gpsimd.dma_start(
                        WfPk[ch][0:HL, :],
                        wfg_d[ch:ch + 1, HH * 6:HID * 6].rearrange("p (k g) -> (p k) g", k=HL))
                    nc.gpsimd.dma_start(
                        WfPk[ch][64:64 + HL, :],
                        wfg_d[ch:ch + 1, 182 * 6 + HH * 6:182 * 6 + HID * 6].rearrange(
                            "p (k g) -> (p k) g", k=HL))
                    nc.gpsimd.dma_start(
                        WfPk[ch][117:118, :],
                        wfg_d[ch:ch + 1, HID * 6:182 * 6])

            # ---------- main pipeline ----------
            ngroups = C // GROUP_SIZE

            def plane(pool, tag, rows, dtp=F16, bufs=None, cols=NPTS):
                return pool.tile([rows, cols], dtp, tag=tag, name=tag, bufs=bufs)

            hplanes = {}

            def emit_l0_exp(chans, grp):
                for ch in chans:
                    j, s = ch // 3, ch % 3
                    b = 32 * s
                    qacc_h = plane(pl2, "qacc_h", HH, bufs=1)
                    qacc_l = plane(pl2, "qacc_l", HL, bufs=1)
                    qtmp_h = plane(pl2, "qtmp_h", HH, bufs=1)
                    qtmp_l = plane(pl2, "qtmp_l", HL, bufs=1)
                    arg_h = plane(pl2, "arg_h", HH, cols=2 * NPTS)
                    arg_l = plane(pl2, "arg_l", HL, cols=2 * NPTS)
                    e_h = plane(pl2, "e_h", HH)
                    e_l = plane(pl2, "e_l", HL)
                    for nt in range(NT):
                        sl = slice(nt * PT, (nt + 1) * PT)
                        sl2 = slice(NPTS + nt * PT, NPTS + (nt + 1) * PT)
                        p_h = pmain.tile([HH, PT], F32, space="PSUM", tag="pa", bufs=2, name="pa")
                        p_l = pmain.tile([HL, PT], F32, space="PSUM", tag="pb", bufs=1, name="pb")
                        p2_h = pmain.tile([HH, PT], F32, space="PSUM", tag="pc", bufs=2, name="pc")
                        p2_l = pmain.tile([HL, PT], F32, space="PSUM", tag="pd", bufs=1, name="pd")
                        rhs = xsb[j][b:b + 3, sl]
                        lh = w0sb[j][b:b + 3, :]
                        nc.tensor.matmul(p_h[:], lh[:, 0:HH], rhs, start=True, stop=True)
                        nc.tensor.matmul(p_l[:], lh[:, HH:HID], rhs, start=True, stop=True)
                        nc.tensor.matmul(p2_h[:], lh[:, 184:312], rhs, start=True, stop=True)
                        nc.tensor.matmul(p2_l[:], lh[:, 312:312 + HL], rhs, start=True, stop=True)
                        nc.scalar.activation(qacc_h[:, sl], p_h[:], AF.Square, scale=1.0 / OM)
                        nc.scalar.activation(qacc_l[:, sl], p_l[:], AF.Square, scale=1.0 / OM)
                        nc.scalar.activation(qtmp_h[:, sl], p2_h[:], AF.Square, scale=1.0 / OM)
                        nc.scalar.activation(qtmp_l[:, sl], p2_l[:], AF.Square, scale=1.0 / OM)
                        nc.vector.add_range_wrap(arg_h[:, sl], p_h[:], 0.0, PI, 2 * PI)
                        nc.vector.add_range_wrap(arg_h[:, sl2], p_h[:], PI / 2, PI, 2 * PI)
                        nc.vector.add_range_wrap(arg_l[:, sl], p_l[:], 0.0, PI, 2 * PI)
                        nc.vector.add_range_wrap(arg_l[:, sl2], p_l[:], PI / 2, PI, 2 * PI)
                        nc.vector.tensor_add(qacc_h[:, sl], qacc_h[:, sl], qtmp_h[:, sl])
                        nc.gpsimd.tensor_add(qacc_l[:, sl], qacc_l[:, sl], qtmp_l[:, sl])
                    nc.scalar.activation(e_h[:], qacc_h[:], AF.Exp, scale=-100.0)
                    nc.scalar.activation(e_l[:], qacc_l[:], AF.Exp, scale=-100.0)
                    grp[ch] = (arg_h, arg_l, e_h, e_l)

            def emit_l0_trig(chans, grp):
                for ch in chans:
                    arg_h, arg_l, e_h, e_l = grp[ch]
                    sc_h = plane(pl2, "sc_h", HH, bufs=1, cols=2 * NPTS)
                    sc_l = plane(pl2, "sc_l", HL, bufs=1, cols=2 * NPTS)
                    h0re_h = plane(pl1, "h0re_h", HH)
                    h0im_h = plane(pl1, "h0im_h", HH)
                    h0lo = plane(pl1, "h0lo", KP)
                    nc.vector.memset(h0lo[:], 1.0)
                    nc.scalar.activation(sc_h[:], arg_h[:], AF.Sin)
                    nc.scalar.activation(sc_l[:], arg_l[:], AF.Sin)
                    nc.vector.tensor_mul(h0re_h[:], e_h[:], sc_h[:, NPTS:])
                    nc.vector.tensor_mul(h0im_h[:], e_h[:], sc_h[:, 0:NPTS])
                    nc.vector.tensor_mul(h0lo[0:HL, :], e_l[:], sc_l[:, NPTS:])
                    nc.vector.tensor_mul(h0lo[64:64 + HL, :], e_l[:], sc_l[:, 0:NPTS])
                    hplanes[ch] = (h0re_h, h0im_h, h0lo)

            def emit_l1_exp(chans, grp1):
                for ch in chans:
                    h0re_h, h0im_h, h0lo = hplanes[ch]
                    qacc_h = plane(pl2, "qacc_h", HH, bufs=1)
                    qacc_l = plane(pl2, "qacc_l", HL, bufs=1)
                    qtmp_h = plane(pl2, "qtmp_h", HH, bufs=1)
                    qtmp_l = plane(pl2, "qtmp_l", HL, bufs=1)
                    arg_h = plane(pl2, "arg_h", HH, cols=2 * NPTS)
                    arg_l = plane(pl2, "arg_l", HL, cols=2 * NPTS)
                    e_h = plane(pl2, "e_h", HH)
                    e_l = plane(pl2, "e_l", HL)

                    def mm3(psum, o, msl, rhs_sl):
                        m_a, m_b = O_MAP[o]
                        nc.tensor.matmul(psum, Whi[ch][m_a][:, msl], h0re_h[:, rhs_sl],
                                         start=True, stop=False)
                        nc.tensor.matmul(psum, Whi[ch][m_b][:, msl], h0im_h[:, rhs_sl],
                                         start=False, stop=False)
                        nc.tensor.matmul(psum, Wpk[ch][o][:, msl], h0lo[:, rhs_sl],
                                         start=False, stop=True)

                    for nt in range(NT):
                        sl = slice(nt * PT, (nt + 1) * PT)
                        sl2 = slice(NPTS + nt * PT, NPTS + (nt + 1) * PT)
                        pre_h = pmain.tile([HH, PT], F32, space="PSUM", tag="pa", bufs=2, name="pa")
                        pre_l = pmain.tile([HL, PT], F32, space="PSUM", tag="pb", bufs=1, name="pb")
                        pim_h = pmain.tile([HH, PT], F32, space="PSUM", tag="pc", bufs=2, name="pc")
                        pim_l = pmain.tile([HL, PT], F32, space="PSUM", tag="pd", bufs=1, name="pd")
                        mm3(pre_h[:], 0, slice(0, HH), sl)
                        mm3(pre_l[:], 0, slice(HH, HID), sl)
                        mm3(pim_h[:], 1, slice(0, HH), sl)
                        mm3(pim_l[:], 1, slice(HH, HID), sl)
                        nc.scalar.activation(qacc_h[:, sl], pre_h[:], AF.Square, scale=1.0 / OM)
                        nc.scalar.activation(qacc_l[:, sl], pre_l[:], AF.Square, scale=1.0 / OM)
                        nc.scalar.activation(qtmp_h[:, sl], pim_h[:], AF.Square,
                                             scale=1.0 / OM, bias=c_015[:HH, :1])
                        nc.scalar.activation(qtmp_l[:, sl], pim_l[:], AF.Square,
                                             scale=1.0 / OM, bias=c_015[:HL, :1])
                        nc.vector.add_range_wrap(arg_h[:, sl], pre_h[:], 0.0, PI, 2 * PI)
                        nc.vector.add_range_wrap(arg_h[:, sl2], pre_h[:], PI / 2, PI, 2 * PI)
                        nc.vector.add_range_wrap(arg_l[:, sl], pre_l[:], 0.0, PI, 2 * PI)
                        nc.vector.add_range_wrap(arg_l[:, sl2], pre_l[:], PI / 2, PI, 2 * PI)
                        nc.vector.tensor_add(qacc_h[:, sl], qacc_h[:, sl], qtmp_h[:, sl])
                        nc.gpsimd.tensor_add(qacc_l[:, sl], qacc_l[:, sl], qtmp_l[:, sl])
                        p2re_h = pmain.tile([HH, PT], F32, space="PSUM", tag="pa", bufs=2, name="pa")
                        p2re_l = pmain.tile([HL, PT], F32, space="PSUM", tag="pb", bufs=1, name="pb")
                        p2im_h = pmain.tile([HH, PT], F32, space="PSUM", tag="pc", bufs=2, name="pc")
                        p2im_l = pmain.tile([HL, PT], F32, space="PSUM", tag="pd", bufs=1, name="pd")
                        mm3(p2re_h[:], 2, slice(0, HH), sl)
                        mm3(p2re_l[:], 2, slice(HH, HID), sl)
                        mm3(p2im_h[:], 3, slice(0, HH), sl)
                        mm3(p2im_l[:], 3, slice(HH, HID), sl)
                        nc.scalar.activation(qtmp_h[:, sl], p2re_h[:], AF.Square, scale=1.0 / OM)
                        nc.scalar.activation(qtmp_l[:, sl], p2re_l[:], AF.Square, scale=1.0 / OM)
                        nc.vector.tensor_add(qacc_h[:, sl], qacc_h[:, sl], qtmp_h[:, sl])
                        nc.gpsimd.tensor_add(qacc_l[:, sl], qacc_l[:, sl], qtmp_l[:, sl])
                        nc.scalar.activation(qtmp_h[:, sl], p2im_h[:], AF.Square, scale=1.0 / OM)
                        nc.scalar.activation(qtmp_l[:, sl], p2im_l[:], AF.Square, scale=1.0 / OM)
                        nc.vector.tensor_add(qacc_h[:, sl], qacc_h[:, sl], qtmp_h[:, sl])
                        nc.gpsimd.tensor_add(qacc_l[:, sl], qacc_l[:, sl], qtmp_l[:, sl])
                    nc.scalar.activation(e_h[:], qacc_h[:], AF.Exp,
                                         scale=-100.0, bias=c_225[:HH, :1])
                    nc.scalar.activation(e_l[:], qacc_l[:], AF.Exp,
                                         scale=-100.0, bias=c_225[:HL, :1])
                    grp1[ch] = (arg_h, arg_l, e_h, e_l)

            def emit_l1_trig(chans, grp1):
                for ch in chans:
                    arg_h, arg_l, e_h, e_l = grp1[ch]
                    sc_h = plane(pl2, "sc_h", HH, bufs=1, cols=2 * NPTS)
                    sc_l = plane(pl2, "sc_l", HL, bufs=1, cols=2 * NPTS)
                    h1re_h = plane(pl1, "h0re_h", HH)
                    h1im_h = plane(pl1, "h0im_h", HH)
                    h1lo = plane(pl1, "h0lo", KP)
                    nc.vector.memset(h1lo[:], 1.0)
                    nc.scalar.activation(sc_h[:], arg_h[:], AF.Sin)
                    nc.scalar.activation(sc_l[:], arg_l[:], AF.Sin)
                    nc.vector.tensor_mul(h1re_h[:], e_h[:], sc_h[:, NPTS:])
                    nc.vector.tensor_mul(h1im_h[:], e_h[:], sc_h[:, 0:NPTS])
                    nc.vector.tensor_mul(h1lo[0:HL, :], e_l[:], sc_l[:, NPTS:])
                    nc.vector.tensor_mul(h1lo[64:64 + HL, :], e_l[:], sc_l[:, 0:NPTS])
                    for nt in range(NT):
                        sl = slice(nt * PT, (nt + 1) * PT)
                        pf = pmain.tile([6, PT], F32, space="PSUM", tag="fin", bufs=1, name="pf")
                        nc.tensor.matmul(pf[:], WfHi[ch][0][:], h1re_h[:, sl], start=True, stop=False)
                        nc.tensor.matmul(pf[:], WfHi[ch][1][:], h1im_h[:, sl], start=False, stop=False)
                        nc.tensor.matmul(pf[:], WfPk[ch][:], h1lo[:, sl], start=False, stop=True)
                        fs = fpool.tile([6, PT], F32, tag="fstage")
                        nc.vector.tensor_copy(fs[:], pf[:])
                        nc.sync.dma_start(out48[6 * ch:6 * ch + 6, sl], fs[:])

            # emission order: g0 L0 -> W-build -> g0 trig/L1/... -> g1 ...
            groups = [list(range(g * GROUP_SIZE, (g + 1) * GROUP_SIZE))
                      for g in range(ngroups)]
            grps = [dict() for _ in range(ngroups)]
            grps1 = [dict() for _ in range(ngroups)]

            emit_l0_exp(groups[0], grps[0])
            emit_wbuild()
            emit_l0_trig(groups[0], grps[0])
            for g in range(ngroups):
                if g > 0:
                    emit_l0_exp(groups[g], grps[g])
                    emit_l0_trig(groups[g], grps[g])
                emit_l1_exp(groups[g], grps1[g])
                emit_l1_trig(groups[g], grps1[g])

    nc.compile()
    return nc


def _prep(inputs):
    x = np.ascontiguousarray(inputs["x"], dtype=np.float32)
    indices = np.ascontiguousarray(inputs["indices"], dtype=np.int32)
    w0_lin = np.asarray(inputs["w0_lin"], dtype=np.float32)
    b0_lin = np.asarray(inputs["b0_lin"], dtype=np.float32)
    w0_orth = np.asarray(inputs["w0_orth"], dtype=np.float32)
    b0_orth = np.asarray(inputs["b0_orth"], dtype=np.float32)
    U1_lin = np.asarray(inputs["U1_lin"], dtype=np.complex64)
    V1_lin = np.asarray(inputs["V1_lin"], dtype=np.complex64)
    b1_lin = np.asarray(inputs["b1_lin"], dtype=np.complex64)
    U1_orth = np.asarray(inputs["U1_orth"], dtype=np.complex64)
    V1_orth = np.asarray(inputs["V1_orth"], dtype=np.complex64)
    b1_orth = np.asarray(inputs["b1_orth"], dtype=np.complex64)
    Wf = np.asarray(inputs["Wf"], dtype=np.complex64)
    bf = np.asarray(inputs["bf"], dtype=np.complex64)

    w0pack = np.zeros((NCH, 3, 368), np.float32)
    w0pack[:, 0:2, 0:HID] = w0_lin
    w0pack[:, 2, 0:HID] = b0_lin[:, 0, :]
    w0pack[:, 0:2, 184:184 + HID] = w0_orth
    w0pack[:, 2, 184:184 + HID] = b0_orth[:, 0, :]
    w0pack = w0pack.reshape(NCH, 3 * 368)

    up = np.concatenate([U1_lin.real, U1_lin.imag, U1_orth.real, U1_orth.imag],
                        axis=1).astype(np.float32)  # (128, 16)
    upack = np.concatenate([up, -up], axis=1)  # (128, 32)
    vpack = np.concatenate([V1_lin.real, V1_lin.imag, V1_orth.real, V1_orth.imag],
                           axis=0).astype(np.float32)  # (16, 32761)
    vpack = np.concatenate([vpack, np.zeros((16, 1), np.float32)], axis=1)
    z = np.zeros_like(b1_lin[:, 0, :].real)
    b1pack = np.concatenate([b1_lin[:, 0, :].real, b1_lin[:, 0, :].imag, z,
                             b1_orth[:, 0, :].real, b1_orth[:, 0, :].imag, z],
                            axis=1).astype(np.float32)  # (128, 1086)

    wfpack = np.zeros((NCH, 2, 182, 6), np.float32)
    wfpack[:, 0, 0:HID, 0:3] = Wf.real
    wfpack[:, 0, 0:HID, 3:6] = Wf.imag
    wfpack[:, 0, HID, 0:3] = bf[:, 0, :].real
    wfpack[:, 0, HID, 3:6] = bf[:, 0, :].imag
    wfpack[:, 1, 0:HID, 0:3] = -Wf.imag
    wfpack[:, 1, 0:HID, 3:6] = Wf.real
    wfpack = wfpack.reshape(NCH, 2 * 182 * 6)

    in_maps = []
    for core in range(NCORES):
        c0 = core * C
        xs = x[c0:c0 + C]
        xpk = [np.zeros((67, NPTS), np.float16) for _ in range(3)]
        for ch in range(C):
            j, s = ch // 3, ch % 3
            xpk[j][32 * s:32 * s + 2, :] = xs[ch].T
            xpk[j][32 * s + 2, :] = 1.0
        m = {f"xpk{j}": xpk[j] for j in range(3)}
        m["idx"] = indices[c0:c0 + C].reshape(C, 1)
        m["w0pack"] = w0pack
        m["upack"] = upack
        m["vpack"] = vpack
        m["b1pack"] = b1pack
        m["wfpack"] = wfpack
        in_maps.append(m)
    return in_maps


def kernel(**inputs):
    from concourse import bass_utils
    if "nc" not in _CACHE:
        _CACHE["nc"] = _build()
    nc = _CACHE["nc"]
    in_maps = _prep(inputs)
    res = bass_utils.run_bass_kernel_spmd(nc, in_maps, core_ids=list(range(NCORES)))
    out = np.zeros((NSEL, NPTS, OUT), np.complex64)
    for core in range(NCORES):
        o = res.results[core]["out48"]
        for ch in range(C):
            re = o[6 * ch:6 * ch + 3, :]
            im = o[6 * ch + 3:6 * ch + 6, :]
            out[core * C + ch] = (re + 1j * im).T.astype(np.complex64)
    return out
